# revision 26
# baseline (speedup 1.0000x reference)
"""3-layer Elman RNN (tanh) Trainium2 kernel.

Model: x(512,2048) int -> emb(27,20) lookup -> RNN 20->32 -> 32->64 -> 64->64
       -> FC 64->26.  Output (512, 2048, 26) f32.

Strategy (per core, batch sharded 8 ways -> 256 batch/core, split into two
ping-pong halves of 128 so ACT and PE overlap across the serial recurrence):

All three layers advance in a skewed pipeline: at macro-step s, layer 1
processes t=s, layer 2 t=s-1, layer 3 t=s-2.  Per half-step one PSUM tile
P[128, 256] holds all three pre-activations:
  P[0:64,  0:128]   = pre2     P[64:128, 0:128] = pre3
  P[64:96, 128:256] = pre1     (rest written zero by the padded matmuls)
filled by 4 matmuls, then ONE ACT tanh op covers the whole tile; layer-2/3
biases ride the ACT per-partition bias vector, layer-1's bias is folded into
the one-hot embedding table (one-hot rows sum to 1).

Every stationary operand is zero-padded to the full [128, 128] array and all
matmuls run WITHOUT tile_position: full-array LDW+MM pairs get the weight
load pipelined into the previous matmul's stream (background weight buffer),
so each pair costs only its ~N/f stream time (~55ns warm) instead of serial
LDW+stream (~215ns).  The resulting dense back-to-back PE stream also keeps
the HAM clock gate at K=8/8 (2.4 GHz) for the whole kernel - the baseline's
sparse tile_position'd matmuls ran at K=4/8.  Padded weight rows multiply
garbage rhs partitions by zero; every such region must be FINITE (NaN*0=NaN),
so the one-hot tiles' unused rows are memset once per pool slot.

The one-hot matmul is the PSUM accumulation-group opener (start=True over all
128 partitions); it has no recurrence dependency, so the PE streams it during
the ACT wait.  FC (lagged 3 steps, reading hprev) runs per half-step straight
out of hn - emitted AFTER the chain matmuls to avoid head-of-line blocking
the PE queue.  FC bias is added by the DVE output copy (tensor_scalar_add).
Output is written [26, T*B] per core and reassembled on host.

Steady state is Scalar-ACT-bound: 2 tanh ops per macro-step x (256+~310
cycles)/1.2GHz ~= 940ns/macro; the PE (10 matmuls/macro) hides underneath.
"""

import os
import sys

sys.path.insert(0, "/opt/trn_rl_repo")

import numpy as np

import concourse.bacc as bacc
import concourse.tile as tile
from concourse import mybir
from concourse.tile_rust import add_dep_helper

T = int(os.environ.get("RNN_T", "512"))  # env override only for debugging
B = 2048
NCORES = 8
BC = B // NCORES          # batch per core = 256
HB = BC // 2              # half-batch = 128
VOCAB, EMB, H1, H2, H3, OUT = 27, 20, 32, 64, 64, 26
S = T + 3                 # macro steps incl. pipeline flush (FC lags 3)

MM_DT = mybir.dt.bfloat16     # matmul operand dtype (states/weights)

import ml_dtypes  # noqa: E402

_NP_OF = {mybir.dt.bfloat16: ml_dtypes.bfloat16, mybir.dt.float32: np.float32}

P1 = 64   # partition base of the pre1/h1 block


def _build_nc():
    nc = bacc.Bacc()
    f32 = mybir.dt.float32
    mdt = MM_DT

    oh_d = nc.dram_tensor("oh", [VOCAB, T * BC], mdt, kind="ExternalInput")
    la_d = nc.dram_tensor("la", [128, 128], mdt, kind="ExternalInput")
    lb_d = nc.dram_tensor("lb", [128, 128], mdt, kind="ExternalInput")
    lc_d = nc.dram_tensor("lc", [128, 128], mdt, kind="ExternalInput")
    le_d = nc.dram_tensor("le", [128, 128], mdt, kind="ExternalInput")
    lf_d = nc.dram_tensor("lf", [128, 128], mdt, kind="ExternalInput")
    b23_d = nc.dram_tensor("b23", [128, 1], f32, kind="ExternalInput")
    bfc_d = nc.dram_tensor("bfc", [OUT, 1], f32, kind="ExternalInput")
    o_d = nc.dram_tensor("o", [OUT, T * BC], f32, kind="ExternalOutput")

    with tile.TileContext(nc) as tc:
        with (
            tc.tile_pool(name="wpool", bufs=1) as wpool,
            tc.tile_pool(name="hpool", bufs=6) as hpool,
            tc.tile_pool(name="ohpool", bufs=3) as ohpool,
            tc.tile_pool(name="opool", bufs=3) as opool,
            tc.tile_pool(name="ppool", bufs=4, space="PSUM") as ppool,
            tc.tile_pool(name="fcpool", bufs=2, space="PSUM") as fcpool,
            tc.tile_pool(name="warmp", bufs=1, space="PSUM") as warmp,
        ):
            # All five stationary operands are zero-padded to the full
            # [128, 128] array: full-array matmuls without tile_position get
            # their LDWEIGHTS pipelined into the previous matmul's stream
            # (background weight buffer), so each LDW+MM pair costs only the
            # ~N/f stream time.  Padded rows multiply garbage rhs partitions
            # by zero; all garbage regions hold finite tanh outputs or zeros.
            la = wpool.tile([128, 128], mdt)   # [Whh2^T Wih3^T; 0 Whh3^T]
            lb = wpool.tile([128, 128], mdt)   # rows 64:96 = W_ih2^T | 0
            lc = wpool.tile([128, 128], mdt)   # rows 64:96 = [0 W_hh1^T 0]
            le = wpool.tile([128, 128], mdt)   # rows 0:27 = [0 EW'' 0]
            lf = wpool.tile([128, 128], mdt)   # rows 64:128 = W_fc^T | 0
            b23 = wpool.tile([128, 1], f32)
            bfc = wpool.tile([OUT, 1], f32)

            zst = wpool.tile([128, 2 * HB], mdt)   # zero initial state
            nc.vector.memset(zst[:], 0.0)
            zoh = wpool.tile([128, HB], mdt)       # zero one-hot, flush steps
            nc.vector.memset(zoh[:], 0.0)

            # One-hot rhs tiles are [128, 8*HB] (4 macro-steps per DMA) with
            # only rows 0:27 DMA'd; rows 27:128 are zeroed once per slot.
            # bufs=1 per tag: each tag is ONE buffer (a tag without bufs=1
            # would rotate `pool.bufs` buffers and the prime-once memset
            # would only ever touch the first -> uninitialized rows 27:128,
            # NaN poison on cores whose SBUF garbage contains NaNs).
            ohslots = []
            for i in range(3):
                t_ = ohpool.tile([128, 8 * HB], mdt, tag=f"oh{i}", bufs=1)
                nc.vector.memset(t_[:], 0.0)
                ohslots.append(t_)

            # DMA order: le + first one-hot chunk first (they gate the loop's
            # opener matmul), then the rest of the weights.
            nc.sync.dma_start(le[:], le_d[:])
            nc.sync.dma_start(ohslots[0][0:VOCAB, :], oh_d[:, 0:8 * HB])
            nc.sync.dma_start(la[:], la_d[:])
            nc.sync.dma_start(lb[:], lb_d[:])
            nc.sync.dma_start(lc[:], lc_d[:])
            nc.sync.dma_start(lf[:], lf_d[:])
            nc.sync.dma_start(b23[:], b23_d[:])
            nc.sync.dma_start(bfc[:], bfc_d[:])

            # PE warmup: back-to-back matmuls trip the HAM clock gate to 8/8
            # (2.4 GHz) and keep it there until the loop's first matmuls are
            # ready (~14us in: weight + first one-hot DMAs).  The profile of
            # the 12-MM version showed HAM re-throttling at 17.5us and the
            # whole loop running at K=4/8 (1.2 GHz).
            warm = wpool.tile([128, 512], mdt)
            nc.vector.memset(warm[:], 0.0)
            wp = warmp.tile([128, 512], mybir.dt.float32)
            for _ in range(30):
                nc.tensor.matmul(wp[:], warm[:, 0:128], warm[:], start=True, stop=True)

            # Prime the ACT tanh table set during the warmup/DMA window;
            # otherwise the first loop ACTIVATE pays a 1.3us ACT_TABLE_LOAD
            # mid-pipeline-fill (and the resulting PE idle re-throttles HAM).
            actprime = wpool.tile([1, 2], f32)
            nc.vector.memset(actprime[:], 0.0)
            nc.scalar.activation(actprime[0:1, 1:2], actprime[0:1, 0:1],
                                 mybir.ActivationFunctionType.Tanh)

            hprev = [zst, zst]
            oht = ohslots[0]
            fco = None
            fco_prev = None
            tanh = mybir.ActivationFunctionType.Tanh

            for s in range(S):
                if s % 4 == 0 and 0 < s < T:
                    g = s // 4
                    oht = ohpool.tile([128, 8 * HB], mdt, tag=f"oh{g % 3}",
                                      bufs=1)
                    nc.sync.dma_start(oht[0:VOCAB, :],
                                      oh_d[:, 8 * HB * g:8 * HB * (g + 1)])
                for half in range(2):
                    hp = hprev[half]
                    if s < T + 2:
                        p = ppool.tile([128, 2 * HB], f32)
                        if s < T:
                            o0 = (s % 4) * 2 * HB + half * HB
                            ohs = oht[:, o0:o0 + HB]
                        else:
                            ohs = zoh[:]
                        # One accumulation group per half-step+bank.  The
                        # opener (start=True) clears has_written for all 128
                        # partitions across the full bank width.  The one-hot
                        # matmul plays opener: it has no recurrence dep, so
                        # the PE streams it during the ACT wait instead of
                        # stalling head-of-line on h(s-1).
                        mm_e = nc.tensor.matmul(p[:, HB:2 * HB], le[:], ohs,
                                                start=True, stop=False,
                                                skip_group_check=True)
                        mm_la = nc.tensor.matmul(p[:, 0:HB], la[:], hp[:, 0:HB],
                                                 start=False, stop=False,
                                                 skip_group_check=True)
                        add_dep_helper(mm_la.ins, mm_e.ins, sync=False,
                                       reason="group opener executes first")
                        nc.tensor.matmul(p[:, 0:HB], lb[:], hp[:, HB:2 * HB],
                                         start=False, stop=False,
                                         skip_group_check=True)
                        nc.tensor.matmul(p[:, HB:2 * HB], lc[:],
                                         hp[:, HB:2 * HB],
                                         start=False, stop=True,
                                         skip_group_check=True)
                    # FC for t3 = s-3 reads hp (= hn of step s-1, whose h3
                    # rows carry layer-3's output for t3).  Emitted AFTER the
                    # chain matmuls: ahead of them it head-of-line blocks the
                    # PE queue on its semaphore wait (measured 616ns stalls on
                    # alternating half-steps).
                    if s >= 3:
                        t3 = s - 3
                        j = 2 * (t3 % 2) + half
                        if j == 0:
                            fco_prev = fco
                            fco = fcpool.tile([128, 4 * HB], f32)
                        nc.tensor.matmul(fco[:, HB * j:HB * (j + 1)],
                                         lf[:], hp[:, 0:HB],
                                         start=True, stop=True,
                                         skip_group_check=True)
                        if j == 3 and (t3 // 2) % 2 == 1:
                            # Ship two 4*HB chunks per DMA to halve the Sync
                            # queue's descriptor load.
                            c = t3 // 2
                            outs = opool.tile([OUT, 8 * HB], f32)
                            nc.vector.tensor_scalar_add(outs[:, 0:4 * HB],
                                                        fco_prev[0:OUT, :],
                                                        bfc[:])
                            nc.vector.tensor_scalar_add(outs[:, 4 * HB:],
                                                        fco[0:OUT, :], bfc[:])
                            nc.sync.dma_start(
                                o_d[:, 4 * HB * (c - 1):4 * HB * (c + 1)],
                                outs[:])
                    if s < T + 2:
                        hn = hpool.tile([128, 2 * HB], mdt)
                        nc.scalar.activation(hn[:], p[:], tanh, bias=b23[:])
                        if s == 0:
                            nc.vector.memset(hn[:, 0:HB], 0.0)      # H2,H3 inv
                        elif s == 1:
                            nc.vector.memset(hn[H2:128, 0:HB], 0.0)  # H3 inv
                        hprev[half] = hn
    nc.compile()
    return nc


_NC_CACHE = None


def _get_nc():
    global _NC_CACHE
    if _NC_CACHE is None:
        _NC_CACHE = _build_nc()
    return _NC_CACHE


def _prep_inputs(inputs):
    npdt = _NP_OF[MM_DT]
    f32 = np.float32
    x = np.asarray(inputs["x"]).astype(np.int64)            # (T, B)
    emb = np.asarray(inputs["emb"], f32)
    W_ih1 = np.asarray(inputs["W_ih1"], f32)
    W_hh1 = np.asarray(inputs["W_hh1"], f32)
    b1 = np.asarray(inputs["b_ih1"], f32) + np.asarray(inputs["b_hh1"], f32)
    W_ih2 = np.asarray(inputs["W_ih2"], f32)
    W_hh2 = np.asarray(inputs["W_hh2"], f32)
    b2 = np.asarray(inputs["b_ih2"], f32) + np.asarray(inputs["b_hh2"], f32)
    W_ih3 = np.asarray(inputs["W_ih3"], f32)
    W_hh3 = np.asarray(inputs["W_hh3"], f32)
    b3 = np.asarray(inputs["b_ih3"], f32) + np.asarray(inputs["b_hh3"], f32)
    W_fc = np.asarray(inputs["W_fc"], f32)
    b_fc = np.asarray(inputs["b_fc"], f32)

    # lhsT blocks (stationary operands), all zero-padded to [128, 128] so
    # every matmul is a full-array LDW+MM pair with pipelined weight load.
    la = np.zeros((128, 128), f32)
    la[0:H2, 0:H2] = W_hh2.T
    la[0:H2, H2:] = W_ih3.T
    la[H2:, H2:] = W_hh3.T
    lb = np.zeros((128, 128), f32)
    lb[P1:P1 + H1, 0:H2] = W_ih2.T
    lc = np.zeros((128, 128), f32)
    lc[P1:P1 + H1, P1:P1 + H1] = W_hh1.T
    # EW'' table: emb @ W_ih1^T + b1, minus the b23[64:96] (= b3[:32]) that
    # the ACT bias vector adds on the pre1 partitions.  Rows 0:27 (one-hot
    # vocab), output cols 64:96 (pre1 partitions).
    ew = emb @ W_ih1.T + b1[None, :] - b3[None, 0:H1]        # [27, 32]
    le = np.zeros((128, 128), f32)
    le[0:VOCAB, P1:P1 + H1] = ew
    lf = np.zeros((128, 128), f32)
    lf[P1:128, 0:OUT] = W_fc.T
    b23 = np.concatenate([b2, b3]).reshape(128, 1).astype(f32)
    bfc = b_fc.reshape(OUT, 1).astype(f32)

    shared = {
        "la": la.astype(npdt), "lb": lb.astype(npdt), "lc": lc.astype(npdt),
        "le": le.astype(npdt), "lf": lf.astype(npdt), "b23": b23, "bfc": bfc,
    }
    in_maps = []
    for core in range(NCORES):
        xc = x[:, core * BC:(core + 1) * BC]                 # (T, BC)
        # one-hot [27, T*BC], free order (t, b)
        oh = (xc.reshape(T * BC)[None, :] == np.arange(VOCAB)[:, None])
        in_maps.append(dict(shared, oh=np.ascontiguousarray(oh.astype(npdt))))
    return in_maps


def _assemble(results):
    cores = []
    for core in range(NCORES):
        o = results[core]["o"]                               # [26, T*BC]
        cores.append(o.reshape(OUT, T, BC).transpose(1, 2, 0))
    return np.ascontiguousarray(np.concatenate(cores, axis=1), dtype=np.float32)


def _run(inputs, **spmd_kwargs):
    """Returns (output, BassKernelResults). spmd_kwargs e.g. trace=True."""
    from concourse.bass_utils import run_bass_kernel_spmd
    nc = _get_nc()
    in_maps = _prep_inputs(inputs)
    res = run_bass_kernel_spmd(nc, in_maps, core_ids=list(range(NCORES)),
                               **spmd_kwargs)
    return _assemble(res.results), res


def kernel(**inputs) -> np.ndarray:
    return _run(inputs)[0]


if __name__ == "__main__":
    import reference as R
    ins = {k: np.asarray(v) for k, v in R.setup_inputs().items()}
    got = kernel(**ins)
    import jax.numpy as jnp
    want = np.asarray(R.reference(**{k: jnp.asarray(v) for k, v in ins.items()}))
    err = np.abs(got - want)
    print("absmax", err.max(), "rel", err.max() / np.abs(want).max())



# revision 27
# speedup vs baseline: 1.0017x; 1.0017x over previous
"""3-layer Elman RNN (tanh) Trainium2 kernel.

Model: x(512,2048) int -> emb(27,20) lookup -> RNN 20->32 -> 32->64 -> 64->64
       -> FC 64->26.  Output (512, 2048, 26) f32.

Strategy (per core, batch sharded 8 ways -> 256 batch/core, split into two
ping-pong halves of 128 so ACT and PE overlap across the serial recurrence):

All three layers advance in a skewed pipeline: at macro-step s, layer 1
processes t=s, layer 2 t=s-1, layer 3 t=s-2.  Per half-step one PSUM tile
P[128, 256] holds all three pre-activations:
  P[0:64,  0:128]   = pre2     P[64:128, 0:128] = pre3
  P[64:96, 128:256] = pre1     (rest written zero by the padded matmuls)
filled by 4 matmuls, then ONE ACT tanh op covers the whole tile; layer-2/3
biases ride the ACT per-partition bias vector, layer-1's bias is folded into
the one-hot embedding table (one-hot rows sum to 1).

Every stationary operand is zero-padded to the full [128, 128] array and all
matmuls run WITHOUT tile_position: full-array LDW+MM pairs get the weight
load pipelined into the previous matmul's stream (background weight buffer),
so each pair costs only its ~N/f stream time (~55ns warm) instead of serial
LDW+stream (~215ns).  The resulting dense back-to-back PE stream also keeps
the HAM clock gate at K=8/8 (2.4 GHz) for the whole kernel - the baseline's
sparse tile_position'd matmuls ran at K=4/8.  Padded weight rows multiply
garbage rhs partitions by zero; every such region must be FINITE (NaN*0=NaN),
so the one-hot tiles' unused rows are memset once per pool slot.

The one-hot matmul is the PSUM accumulation-group opener (start=True over all
128 partitions); it has no recurrence dependency, so the PE streams it during
the ACT wait.  FC (lagged 3 steps, reading hprev) runs per half-step straight
out of hn - emitted AFTER the chain matmuls to avoid head-of-line blocking
the PE queue.  FC bias is added by the DVE output copy (tensor_scalar_add).
Output is written [26, T*B] per core and reassembled on host.

Steady state is Scalar-ACT-bound: 2 tanh ops per macro-step x (256+~310
cycles)/1.2GHz ~= 940ns/macro; the PE (10 matmuls/macro) hides underneath.
"""

import os
import sys

sys.path.insert(0, "/opt/trn_rl_repo")

import numpy as np

import concourse.bacc as bacc
import concourse.tile as tile
from concourse import mybir
from concourse.tile_rust import add_dep_helper

T = int(os.environ.get("RNN_T", "512"))  # env override only for debugging
B = 2048
NCORES = 8
BC = B // NCORES          # batch per core = 256
HB = BC // 2              # half-batch = 128
VOCAB, EMB, H1, H2, H3, OUT = 27, 20, 32, 64, 64, 26
S = T + 3                 # macro steps incl. pipeline flush (FC lags 3)

MM_DT = mybir.dt.bfloat16     # matmul operand dtype (states/weights)

import ml_dtypes  # noqa: E402

_NP_OF = {mybir.dt.bfloat16: ml_dtypes.bfloat16, mybir.dt.float32: np.float32}

P1 = 64   # partition base of the pre1/h1 block


def _build_nc():
    nc = bacc.Bacc()
    f32 = mybir.dt.float32
    mdt = MM_DT

    oh_d = nc.dram_tensor("oh", [VOCAB, T * BC], mdt, kind="ExternalInput")
    la_d = nc.dram_tensor("la", [128, 128], mdt, kind="ExternalInput")
    lb_d = nc.dram_tensor("lb", [128, 128], mdt, kind="ExternalInput")
    lc_d = nc.dram_tensor("lc", [128, 128], mdt, kind="ExternalInput")
    le_d = nc.dram_tensor("le", [128, 128], mdt, kind="ExternalInput")
    lf_d = nc.dram_tensor("lf", [128, 128], mdt, kind="ExternalInput")
    b23_d = nc.dram_tensor("b23", [128, 1], f32, kind="ExternalInput")
    bfc_d = nc.dram_tensor("bfc", [OUT, 1], f32, kind="ExternalInput")
    o_d = nc.dram_tensor("o", [OUT, T * BC], f32, kind="ExternalOutput")

    with tile.TileContext(nc) as tc:
        with (
            tc.tile_pool(name="wpool", bufs=1) as wpool,
            tc.tile_pool(name="hpool", bufs=6) as hpool,
            tc.tile_pool(name="ohpool", bufs=3) as ohpool,
            tc.tile_pool(name="opool", bufs=3) as opool,
            tc.tile_pool(name="ppool", bufs=4, space="PSUM") as ppool,
            tc.tile_pool(name="fcpool", bufs=2, space="PSUM") as fcpool,
            tc.tile_pool(name="warmp", bufs=1, space="PSUM") as warmp,
        ):
            # All five stationary operands are zero-padded to the full
            # [128, 128] array: full-array matmuls without tile_position get
            # their LDWEIGHTS pipelined into the previous matmul's stream
            # (background weight buffer), so each LDW+MM pair costs only the
            # ~N/f stream time.  Padded rows multiply garbage rhs partitions
            # by zero; all garbage regions hold finite tanh outputs or zeros.
            la = wpool.tile([128, 128], mdt)   # [Whh2^T Wih3^T; 0 Whh3^T]
            lb = wpool.tile([128, 128], mdt)   # rows 64:96 = W_ih2^T | 0
            lc = wpool.tile([128, 128], mdt)   # rows 64:96 = [0 W_hh1^T 0]
            le = wpool.tile([128, 128], mdt)   # rows 0:27 = [0 EW'' 0]
            lf = wpool.tile([128, 128], mdt)   # rows 64:128 = W_fc^T | 0
            b23 = wpool.tile([128, 1], f32)
            bfc = wpool.tile([OUT, 1], f32)

            zst = wpool.tile([128, 2 * HB], mdt)   # zero initial state
            nc.vector.memset(zst[:], 0.0)
            zoh = wpool.tile([128, HB], mdt)       # zero one-hot, flush steps
            nc.vector.memset(zoh[:], 0.0)

            # One-hot rhs tiles are [128, 8*HB] (4 macro-steps per DMA) with
            # only rows 0:27 DMA'd; rows 27:128 are zeroed once per slot.
            # bufs=1 per tag: each tag is ONE buffer (a tag without bufs=1
            # would rotate `pool.bufs` buffers and the prime-once memset
            # would only ever touch the first -> uninitialized rows 27:128,
            # NaN poison on cores whose SBUF garbage contains NaNs).
            ohslots = []
            for i in range(3):
                t_ = ohpool.tile([128, 8 * HB], mdt, tag=f"oh{i}", bufs=1)
                nc.vector.memset(t_[:], 0.0)
                ohslots.append(t_)

            # DMA order: le + first one-hot chunk first (they gate the loop's
            # opener matmul), then the rest of the weights.
            nc.sync.dma_start(le[:], le_d[:])
            nc.sync.dma_start(ohslots[0][0:VOCAB, :], oh_d[:, 0:8 * HB])
            nc.sync.dma_start(la[:], la_d[:])
            nc.sync.dma_start(lb[:], lb_d[:])
            nc.sync.dma_start(lc[:], lc_d[:])
            nc.sync.dma_start(lf[:], lf_d[:])
            nc.sync.dma_start(b23[:], b23_d[:])
            nc.sync.dma_start(bfc[:], bfc_d[:])

            # PE warmup: back-to-back matmuls trip the HAM clock gate to 8/8
            # (2.4 GHz) and keep it there until the loop's first matmuls are
            # ready (~14us in: weight + first one-hot DMAs).  The profile of
            # the 12-MM version showed HAM re-throttling at 17.5us and the
            # whole loop running at K=4/8 (1.2 GHz).
            warm = wpool.tile([128, 512], mdt)
            nc.vector.memset(warm[:], 0.0)
            wp = warmp.tile([128, 512], mybir.dt.float32)
            for _ in range(30):
                nc.tensor.matmul(wp[:], warm[:, 0:128], warm[:], start=True, stop=True)

            # Prime the ACT tanh table set during the warmup/DMA window;
            # otherwise the first loop ACTIVATE pays a 1.3us ACT_TABLE_LOAD
            # mid-pipeline-fill (and the resulting PE idle re-throttles HAM).
            actprime = wpool.tile([1, 2], f32)
            nc.vector.memset(actprime[:], 0.0)
            nc.scalar.activation(actprime[0:1, 1:2], actprime[0:1, 0:1],
                                 mybir.ActivationFunctionType.Tanh)

            hprev = [zst, zst]
            oht = ohslots[0]
            fco = None
            fco_prev = None
            tanh = mybir.ActivationFunctionType.Tanh

            for s in range(S):
                if s % 4 == 0 and 0 < s < T:
                    g = s // 4
                    oht = ohpool.tile([128, 8 * HB], mdt, tag=f"oh{g % 3}",
                                      bufs=1)
                    nc.sync.dma_start(oht[0:VOCAB, :],
                                      oh_d[:, 8 * HB * g:8 * HB * (g + 1)])
                for half in range(2):
                    hp = hprev[half]
                    if s < T + 2:
                        p = ppool.tile([128, 2 * HB], f32)
                        if s < T:
                            o0 = (s % 4) * 2 * HB + half * HB
                            ohs = oht[:, o0:o0 + HB]
                        else:
                            ohs = zoh[:]
                        # One accumulation group per half-step+bank.  The
                        # opener (start=True) clears has_written for all 128
                        # partitions across the full bank width.  The one-hot
                        # matmul plays opener: it has no recurrence dep, so
                        # the PE streams it during the ACT wait instead of
                        # stalling head-of-line on h(s-1).
                        mm_e = nc.tensor.matmul(p[:, HB:2 * HB], le[:], ohs,
                                                start=True, stop=False,
                                                skip_group_check=True)
                        mm_la = nc.tensor.matmul(p[:, 0:HB], la[:], hp[:, 0:HB],
                                                 start=False, stop=False,
                                                 skip_group_check=True)
                        add_dep_helper(mm_la.ins, mm_e.ins, sync=False,
                                       reason="group opener executes first")
                        nc.tensor.matmul(p[:, 0:HB], lb[:], hp[:, HB:2 * HB],
                                         start=False, stop=False,
                                         skip_group_check=True)
                        nc.tensor.matmul(p[:, HB:2 * HB], lc[:],
                                         hp[:, HB:2 * HB],
                                         start=False, stop=True,
                                         skip_group_check=True)
                    # FC for t3 = s-3 reads hp (= hn of step s-1, whose h3
                    # rows carry layer-3's output for t3).  Emitted AFTER the
                    # chain matmuls: ahead of them it head-of-line blocks the
                    # PE queue on its semaphore wait (measured 616ns stalls on
                    # alternating half-steps).
                    if s >= 3:
                        t3 = s - 3
                        j = 2 * (t3 % 2) + half
                        if j == 0:
                            fco_prev = fco
                            fco = fcpool.tile([128, 4 * HB], f32)
                        nc.tensor.matmul(fco[:, HB * j:HB * (j + 1)],
                                         lf[:], hp[:, 0:HB],
                                         start=True, stop=True,
                                         skip_group_check=True)
                        if j == 3:
                            c = t3 // 2
                            outs = opool.tile([OUT, 4 * HB], f32)
                            nc.vector.tensor_scalar_add(outs[:],
                                                        fco[0:OUT, :], bfc[:])
                            nc.sync.dma_start(o_d[:, 4 * HB * c:4 * HB * (c + 1)],
                                              outs[:])
                    if s < T + 2:
                        hn = hpool.tile([128, 2 * HB], mdt)
                        nc.scalar.activation(hn[:], p[:], tanh, bias=b23[:])
                        if s == 0:
                            nc.vector.memset(hn[:, 0:HB], 0.0)      # H2,H3 inv
                        elif s == 1:
                            nc.vector.memset(hn[H2:128, 0:HB], 0.0)  # H3 inv
                        hprev[half] = hn
    nc.compile()
    return nc


_NC_CACHE = None


def _get_nc():
    global _NC_CACHE
    if _NC_CACHE is None:
        _NC_CACHE = _build_nc()
    return _NC_CACHE


def _prep_inputs(inputs):
    npdt = _NP_OF[MM_DT]
    f32 = np.float32
    x = np.asarray(inputs["x"]).astype(np.int64)            # (T, B)
    emb = np.asarray(inputs["emb"], f32)
    W_ih1 = np.asarray(inputs["W_ih1"], f32)
    W_hh1 = np.asarray(inputs["W_hh1"], f32)
    b1 = np.asarray(inputs["b_ih1"], f32) + np.asarray(inputs["b_hh1"], f32)
    W_ih2 = np.asarray(inputs["W_ih2"], f32)
    W_hh2 = np.asarray(inputs["W_hh2"], f32)
    b2 = np.asarray(inputs["b_ih2"], f32) + np.asarray(inputs["b_hh2"], f32)
    W_ih3 = np.asarray(inputs["W_ih3"], f32)
    W_hh3 = np.asarray(inputs["W_hh3"], f32)
    b3 = np.asarray(inputs["b_ih3"], f32) + np.asarray(inputs["b_hh3"], f32)
    W_fc = np.asarray(inputs["W_fc"], f32)
    b_fc = np.asarray(inputs["b_fc"], f32)

    # lhsT blocks (stationary operands), all zero-padded to [128, 128] so
    # every matmul is a full-array LDW+MM pair with pipelined weight load.
    la = np.zeros((128, 128), f32)
    la[0:H2, 0:H2] = W_hh2.T
    la[0:H2, H2:] = W_ih3.T
    la[H2:, H2:] = W_hh3.T
    lb = np.zeros((128, 128), f32)
    lb[P1:P1 + H1, 0:H2] = W_ih2.T
    lc = np.zeros((128, 128), f32)
    lc[P1:P1 + H1, P1:P1 + H1] = W_hh1.T
    # EW'' table: emb @ W_ih1^T + b1, minus the b23[64:96] (= b3[:32]) that
    # the ACT bias vector adds on the pre1 partitions.  Rows 0:27 (one-hot
    # vocab), output cols 64:96 (pre1 partitions).
    ew = emb @ W_ih1.T + b1[None, :] - b3[None, 0:H1]        # [27, 32]
    le = np.zeros((128, 128), f32)
    le[0:VOCAB, P1:P1 + H1] = ew
    lf = np.zeros((128, 128), f32)
    lf[P1:128, 0:OUT] = W_fc.T
    b23 = np.concatenate([b2, b3]).reshape(128, 1).astype(f32)
    bfc = b_fc.reshape(OUT, 1).astype(f32)

    shared = {
        "la": la.astype(npdt), "lb": lb.astype(npdt), "lc": lc.astype(npdt),
        "le": le.astype(npdt), "lf": lf.astype(npdt), "b23": b23, "bfc": bfc,
    }
    in_maps = []
    for core in range(NCORES):
        xc = x[:, core * BC:(core + 1) * BC]                 # (T, BC)
        # one-hot [27, T*BC], free order (t, b)
        oh = (xc.reshape(T * BC)[None, :] == np.arange(VOCAB)[:, None])
        in_maps.append(dict(shared, oh=np.ascontiguousarray(oh.astype(npdt))))
    return in_maps


def _assemble(results):
    cores = []
    for core in range(NCORES):
        o = results[core]["o"]                               # [26, T*BC]
        cores.append(o.reshape(OUT, T, BC).transpose(1, 2, 0))
    return np.ascontiguousarray(np.concatenate(cores, axis=1), dtype=np.float32)


def _run(inputs, **spmd_kwargs):
    """Returns (output, BassKernelResults). spmd_kwargs e.g. trace=True."""
    from concourse.bass_utils import run_bass_kernel_spmd
    nc = _get_nc()
    in_maps = _prep_inputs(inputs)
    res = run_bass_kernel_spmd(nc, in_maps, core_ids=list(range(NCORES)),
                               **spmd_kwargs)
    return _assemble(res.results), res


def kernel(**inputs) -> np.ndarray:
    return _run(inputs)[0]


if __name__ == "__main__":
    import reference as R
    ins = {k: np.asarray(v) for k, v in R.setup_inputs().items()}
    got = kernel(**ins)
    import jax.numpy as jnp
    want = np.asarray(R.reference(**{k: jnp.asarray(v) for k, v in ins.items()}))
    err = np.abs(got - want)
    print("absmax", err.max(), "rel", err.max() / np.abs(want).max())



# revision 28
# speedup vs baseline: 1.0032x; 1.0016x over previous
"""3-layer Elman RNN (tanh) Trainium2 kernel.

Model: x(512,2048) int -> emb(27,20) lookup -> RNN 20->32 -> 32->64 -> 64->64
       -> FC 64->26.  Output (512, 2048, 26) f32.

Strategy (per core, batch sharded 8 ways -> 256 batch/core, split into two
ping-pong halves of 128 so ACT and PE overlap across the serial recurrence):

All three layers advance in a skewed pipeline: at macro-step s, layer 1
processes t=s, layer 2 t=s-1, layer 3 t=s-2.  Per half-step one PSUM tile
P[128, 256] holds all three pre-activations:
  P[0:64,  0:128]   = pre2     P[64:128, 0:128] = pre3
  P[64:96, 128:256] = pre1     (rest written zero by the padded matmuls)
filled by 4 matmuls, then ONE ACT tanh op covers the whole tile; layer-2/3
biases ride the ACT per-partition bias vector, layer-1's bias is folded into
the one-hot embedding table (one-hot rows sum to 1).

Every stationary operand is zero-padded to the full [128, 128] array and all
matmuls run WITHOUT tile_position: full-array LDW+MM pairs get the weight
load pipelined into the previous matmul's stream (background weight buffer),
so each pair costs only its ~N/f stream time (~55ns warm) instead of serial
LDW+stream (~215ns).  The resulting dense back-to-back PE stream also keeps
the HAM clock gate at K=8/8 (2.4 GHz) for the whole kernel - the baseline's
sparse tile_position'd matmuls ran at K=4/8.  Padded weight rows multiply
garbage rhs partitions by zero; every such region must be FINITE (NaN*0=NaN),
so the one-hot tiles' unused rows are memset once per pool slot.

The one-hot matmul is the PSUM accumulation-group opener (start=True over all
128 partitions); it has no recurrence dependency, so the PE streams it during
the ACT wait.  FC (lagged 3 steps, reading hprev) runs per half-step straight
out of hn - emitted AFTER the chain matmuls to avoid head-of-line blocking
the PE queue.  FC bias is added by the DVE output copy (tensor_scalar_add).
Output is written [26, T*B] per core and reassembled on host.

Steady state is Scalar-ACT-bound: 2 tanh ops per macro-step x (256+~310
cycles)/1.2GHz ~= 940ns/macro; the PE (10 matmuls/macro) hides underneath.
"""

import os
import sys

sys.path.insert(0, "/opt/trn_rl_repo")

import numpy as np

import concourse.bacc as bacc
import concourse.tile as tile
from concourse import mybir
from concourse.tile_rust import add_dep_helper

T = int(os.environ.get("RNN_T", "512"))  # env override only for debugging
B = 2048
NCORES = 8
BC = B // NCORES          # batch per core = 256
HB = BC // 2              # half-batch = 128
VOCAB, EMB, H1, H2, H3, OUT = 27, 20, 32, 64, 64, 26
S = T + 3                 # macro steps incl. pipeline flush (FC lags 3)

MM_DT = mybir.dt.bfloat16     # matmul operand dtype (states/weights)

import ml_dtypes  # noqa: E402

_NP_OF = {mybir.dt.bfloat16: ml_dtypes.bfloat16, mybir.dt.float32: np.float32}

P1 = 64   # partition base of the pre1/h1 block


def _build_nc():
    nc = bacc.Bacc()
    f32 = mybir.dt.float32
    mdt = MM_DT

    oh_d = nc.dram_tensor("oh", [VOCAB, T * BC], mdt, kind="ExternalInput")
    la_d = nc.dram_tensor("la", [128, 128], mdt, kind="ExternalInput")
    lb_d = nc.dram_tensor("lb", [128, 128], mdt, kind="ExternalInput")
    lc_d = nc.dram_tensor("lc", [128, 128], mdt, kind="ExternalInput")
    le_d = nc.dram_tensor("le", [128, 128], mdt, kind="ExternalInput")
    lf_d = nc.dram_tensor("lf", [128, 128], mdt, kind="ExternalInput")
    b23_d = nc.dram_tensor("b23", [128, 1], f32, kind="ExternalInput")
    bfc_d = nc.dram_tensor("bfc", [OUT, 1], f32, kind="ExternalInput")
    o_d = nc.dram_tensor("o", [OUT, T * BC], f32, kind="ExternalOutput")

    with tile.TileContext(nc) as tc:
        with (
            tc.tile_pool(name="wpool", bufs=1) as wpool,
            tc.tile_pool(name="hpool", bufs=6) as hpool,
            tc.tile_pool(name="ohpool", bufs=3) as ohpool,
            tc.tile_pool(name="opool", bufs=3) as opool,
            tc.tile_pool(name="ppool", bufs=4, space="PSUM") as ppool,
            tc.tile_pool(name="fcpool", bufs=2, space="PSUM") as fcpool,
            tc.tile_pool(name="warmp", bufs=1, space="PSUM") as warmp,
        ):
            # All five stationary operands are zero-padded to the full
            # [128, 128] array: full-array matmuls without tile_position get
            # their LDWEIGHTS pipelined into the previous matmul's stream
            # (background weight buffer), so each LDW+MM pair costs only the
            # ~N/f stream time.  Padded rows multiply garbage rhs partitions
            # by zero; all garbage regions hold finite tanh outputs or zeros.
            la = wpool.tile([128, 128], mdt)   # [Whh2^T Wih3^T; 0 Whh3^T]
            lb = wpool.tile([128, 128], mdt)   # rows 64:96 = W_ih2^T | 0
            lc = wpool.tile([128, 128], mdt)   # rows 64:96 = [0 W_hh1^T 0]
            le = wpool.tile([128, 128], mdt)   # rows 0:27 = [0 EW'' 0]
            lf = wpool.tile([128, 128], mdt)   # rows 64:128 = W_fc^T | 0
            b23 = wpool.tile([128, 1], f32)
            bfc = wpool.tile([OUT, 1], f32)

            zst = wpool.tile([128, 2 * HB], mdt)   # zero initial state
            nc.vector.memset(zst[:], 0.0)
            zoh = wpool.tile([128, HB], mdt)       # zero one-hot, flush steps
            nc.vector.memset(zoh[:], 0.0)

            # One-hot rhs tiles are [128, 8*HB] (4 macro-steps per DMA) with
            # only rows 0:27 DMA'd; rows 27:128 are zeroed once per slot.
            # bufs=1 per tag: each tag is ONE buffer (a tag without bufs=1
            # would rotate `pool.bufs` buffers and the prime-once memset
            # would only ever touch the first -> uninitialized rows 27:128,
            # NaN poison on cores whose SBUF garbage contains NaNs).
            ohslots = []
            for i in range(3):
                t_ = ohpool.tile([128, 8 * HB], mdt, tag=f"oh{i}", bufs=1)
                nc.vector.memset(t_[:], 0.0)
                ohslots.append(t_)

            # DMA order: le + first one-hot chunk first (they gate the loop's
            # opener matmul), then the rest of the weights.
            nc.sync.dma_start(le[:], le_d[:])
            nc.sync.dma_start(ohslots[0][0:VOCAB, :], oh_d[:, 0:8 * HB])
            nc.sync.dma_start(la[:], la_d[:])
            nc.sync.dma_start(lb[:], lb_d[:])
            nc.sync.dma_start(lc[:], lc_d[:])
            nc.sync.dma_start(lf[:], lf_d[:])
            nc.sync.dma_start(b23[:], b23_d[:])
            nc.sync.dma_start(bfc[:], bfc_d[:])

            # PE warmup: back-to-back matmuls trip the HAM clock gate to 8/8
            # (2.4 GHz) and keep it there until the loop's first matmuls are
            # ready (~14us in: weight + first one-hot DMAs).  The profile of
            # the 12-MM version showed HAM re-throttling at 17.5us and the
            # whole loop running at K=4/8 (1.2 GHz).
            warm = wpool.tile([128, 512], mdt)
            nc.vector.memset(warm[:], 0.0)
            wp = warmp.tile([128, 512], mybir.dt.float32)
            for _ in range(22):
                nc.tensor.matmul(wp[:], warm[:, 0:128], warm[:], start=True, stop=True)

            # Prime the ACT tanh table set during the warmup/DMA window;
            # otherwise the first loop ACTIVATE pays a 1.3us ACT_TABLE_LOAD
            # mid-pipeline-fill (and the resulting PE idle re-throttles HAM).
            actprime = wpool.tile([1, 2], f32)
            nc.vector.memset(actprime[:], 0.0)
            nc.scalar.activation(actprime[0:1, 1:2], actprime[0:1, 0:1],
                                 mybir.ActivationFunctionType.Tanh)

            hprev = [zst, zst]
            oht = ohslots[0]
            fco = None
            fco_prev = None
            tanh = mybir.ActivationFunctionType.Tanh

            for s in range(S):
                if s % 4 == 0 and 0 < s < T:
                    g = s // 4
                    oht = ohpool.tile([128, 8 * HB], mdt, tag=f"oh{g % 3}",
                                      bufs=1)
                    nc.sync.dma_start(oht[0:VOCAB, :],
                                      oh_d[:, 8 * HB * g:8 * HB * (g + 1)])
                for half in range(2):
                    hp = hprev[half]
                    if s < T + 2:
                        p = ppool.tile([128, 2 * HB], f32)
                        if s < T:
                            o0 = (s % 4) * 2 * HB + half * HB
                            ohs = oht[:, o0:o0 + HB]
                        else:
                            ohs = zoh[:]
                        # One accumulation group per half-step+bank.  The
                        # opener (start=True) clears has_written for all 128
                        # partitions across the full bank width.  The one-hot
                        # matmul plays opener: it has no recurrence dep, so
                        # the PE streams it during the ACT wait instead of
                        # stalling head-of-line on h(s-1).
                        mm_e = nc.tensor.matmul(p[:, HB:2 * HB], le[:], ohs,
                                                start=True, stop=False,
                                                skip_group_check=True)
                        mm_la = nc.tensor.matmul(p[:, 0:HB], la[:], hp[:, 0:HB],
                                                 start=False, stop=False,
                                                 skip_group_check=True)
                        add_dep_helper(mm_la.ins, mm_e.ins, sync=False,
                                       reason="group opener executes first")
                        nc.tensor.matmul(p[:, 0:HB], lb[:], hp[:, HB:2 * HB],
                                         start=False, stop=False,
                                         skip_group_check=True)
                        nc.tensor.matmul(p[:, HB:2 * HB], lc[:],
                                         hp[:, HB:2 * HB],
                                         start=False, stop=True,
                                         skip_group_check=True)
                    # FC for t3 = s-3 reads hp (= hn of step s-1, whose h3
                    # rows carry layer-3's output for t3).  Emitted AFTER the
                    # chain matmuls: ahead of them it head-of-line blocks the
                    # PE queue on its semaphore wait (measured 616ns stalls on
                    # alternating half-steps).
                    if s >= 3:
                        t3 = s - 3
                        j = 2 * (t3 % 2) + half
                        if j == 0:
                            fco_prev = fco
                            fco = fcpool.tile([128, 4 * HB], f32)
                        nc.tensor.matmul(fco[:, HB * j:HB * (j + 1)],
                                         lf[:], hp[:, 0:HB],
                                         start=True, stop=True,
                                         skip_group_check=True)
                        if j == 3:
                            c = t3 // 2
                            outs = opool.tile([OUT, 4 * HB], f32)
                            nc.vector.tensor_scalar_add(outs[:],
                                                        fco[0:OUT, :], bfc[:])
                            nc.sync.dma_start(o_d[:, 4 * HB * c:4 * HB * (c + 1)],
                                              outs[:])
                    if s < T + 2:
                        hn = hpool.tile([128, 2 * HB], mdt)
                        nc.scalar.activation(hn[:], p[:], tanh, bias=b23[:])
                        if s == 0:
                            nc.vector.memset(hn[:, 0:HB], 0.0)      # H2,H3 inv
                        elif s == 1:
                            nc.vector.memset(hn[H2:128, 0:HB], 0.0)  # H3 inv
                        hprev[half] = hn
    nc.compile()
    return nc


_NC_CACHE = None


def _get_nc():
    global _NC_CACHE
    if _NC_CACHE is None:
        _NC_CACHE = _build_nc()
    return _NC_CACHE


def _prep_inputs(inputs):
    npdt = _NP_OF[MM_DT]
    f32 = np.float32
    x = np.asarray(inputs["x"]).astype(np.int64)            # (T, B)
    emb = np.asarray(inputs["emb"], f32)
    W_ih1 = np.asarray(inputs["W_ih1"], f32)
    W_hh1 = np.asarray(inputs["W_hh1"], f32)
    b1 = np.asarray(inputs["b_ih1"], f32) + np.asarray(inputs["b_hh1"], f32)
    W_ih2 = np.asarray(inputs["W_ih2"], f32)
    W_hh2 = np.asarray(inputs["W_hh2"], f32)
    b2 = np.asarray(inputs["b_ih2"], f32) + np.asarray(inputs["b_hh2"], f32)
    W_ih3 = np.asarray(inputs["W_ih3"], f32)
    W_hh3 = np.asarray(inputs["W_hh3"], f32)
    b3 = np.asarray(inputs["b_ih3"], f32) + np.asarray(inputs["b_hh3"], f32)
    W_fc = np.asarray(inputs["W_fc"], f32)
    b_fc = np.asarray(inputs["b_fc"], f32)

    # lhsT blocks (stationary operands), all zero-padded to [128, 128] so
    # every matmul is a full-array LDW+MM pair with pipelined weight load.
    la = np.zeros((128, 128), f32)
    la[0:H2, 0:H2] = W_hh2.T
    la[0:H2, H2:] = W_ih3.T
    la[H2:, H2:] = W_hh3.T
    lb = np.zeros((128, 128), f32)
    lb[P1:P1 + H1, 0:H2] = W_ih2.T
    lc = np.zeros((128, 128), f32)
    lc[P1:P1 + H1, P1:P1 + H1] = W_hh1.T
    # EW'' table: emb @ W_ih1^T + b1, minus the b23[64:96] (= b3[:32]) that
    # the ACT bias vector adds on the pre1 partitions.  Rows 0:27 (one-hot
    # vocab), output cols 64:96 (pre1 partitions).
    ew = emb @ W_ih1.T + b1[None, :] - b3[None, 0:H1]        # [27, 32]
    le = np.zeros((128, 128), f32)
    le[0:VOCAB, P1:P1 + H1] = ew
    lf = np.zeros((128, 128), f32)
    lf[P1:128, 0:OUT] = W_fc.T
    b23 = np.concatenate([b2, b3]).reshape(128, 1).astype(f32)
    bfc = b_fc.reshape(OUT, 1).astype(f32)

    shared = {
        "la": la.astype(npdt), "lb": lb.astype(npdt), "lc": lc.astype(npdt),
        "le": le.astype(npdt), "lf": lf.astype(npdt), "b23": b23, "bfc": bfc,
    }
    in_maps = []
    for core in range(NCORES):
        xc = x[:, core * BC:(core + 1) * BC]                 # (T, BC)
        # one-hot [27, T*BC], free order (t, b)
        oh = (xc.reshape(T * BC)[None, :] == np.arange(VOCAB)[:, None])
        in_maps.append(dict(shared, oh=np.ascontiguousarray(oh.astype(npdt))))
    return in_maps


def _assemble(results):
    cores = []
    for core in range(NCORES):
        o = results[core]["o"]                               # [26, T*BC]
        cores.append(o.reshape(OUT, T, BC).transpose(1, 2, 0))
    return np.ascontiguousarray(np.concatenate(cores, axis=1), dtype=np.float32)


def _run(inputs, **spmd_kwargs):
    """Returns (output, BassKernelResults). spmd_kwargs e.g. trace=True."""
    from concourse.bass_utils import run_bass_kernel_spmd
    nc = _get_nc()
    in_maps = _prep_inputs(inputs)
    res = run_bass_kernel_spmd(nc, in_maps, core_ids=list(range(NCORES)),
                               **spmd_kwargs)
    return _assemble(res.results), res


def kernel(**inputs) -> np.ndarray:
    return _run(inputs)[0]


if __name__ == "__main__":
    import reference as R
    ins = {k: np.asarray(v) for k, v in R.setup_inputs().items()}
    got = kernel(**ins)
    import jax.numpy as jnp
    want = np.asarray(R.reference(**{k: jnp.asarray(v) for k, v in ins.items()}))
    err = np.abs(got - want)
    print("absmax", err.max(), "rel", err.max() / np.abs(want).max())



# revision 30
# speedup vs baseline: 1.0081x; 1.0049x over previous
"""3-layer Elman RNN (tanh) Trainium2 kernel.

Model: x(512,2048) int -> emb(27,20) lookup -> RNN 20->32 -> 32->64 -> 64->64
       -> FC 64->26.  Output (512, 2048, 26) f32.

Strategy (per core, batch sharded 8 ways -> 256 batch/core, split into two
ping-pong halves of 128 so ACT and PE overlap across the serial recurrence):

All three layers advance in a skewed pipeline: at macro-step s, layer 1
processes t=s, layer 2 t=s-1, layer 3 t=s-2.  Per half-step one PSUM tile
P[128, 256] holds all three pre-activations:
  P[0:64,  0:128]   = pre2     P[64:128, 0:128] = pre3
  P[64:96, 128:256] = pre1     (rest written zero by the padded matmuls)
filled by 4 matmuls, then ONE ACT tanh op covers the whole tile; layer-2/3
biases ride the ACT per-partition bias vector, layer-1's bias is folded into
the one-hot embedding table (one-hot rows sum to 1).

Every stationary operand is zero-padded to the full [128, 128] array and all
matmuls run WITHOUT tile_position: full-array LDW+MM pairs get the weight
load pipelined into the previous matmul's stream (background weight buffer),
so each pair costs only its ~N/f stream time (~55ns warm) instead of serial
LDW+stream (~215ns).  The resulting dense back-to-back PE stream also keeps
the HAM clock gate at K=8/8 (2.4 GHz) for the whole kernel - the baseline's
sparse tile_position'd matmuls ran at K=4/8.  Padded weight rows multiply
garbage rhs partitions by zero; every such region must be FINITE (NaN*0=NaN),
so the one-hot tiles' unused rows are memset once per pool slot.

The one-hot matmul is the PSUM accumulation-group opener (start=True over all
128 partitions); it has no recurrence dependency, so the PE streams it during
the ACT wait.  FC (lagged 3 steps, reading hprev) runs per half-step straight
out of hn - emitted AFTER the chain matmuls to avoid head-of-line blocking
the PE queue.  FC bias is added by the DVE output copy (tensor_scalar_add).
Output is written [26, T*B] per core and reassembled on host.

Steady state is Scalar-ACT-bound: 2 tanh ops per macro-step x (256+~310
cycles)/1.2GHz ~= 940ns/macro; the PE (10 matmuls/macro) hides underneath.
"""

import os
import sys

sys.path.insert(0, "/opt/trn_rl_repo")

import numpy as np

import concourse.bacc as bacc
import concourse.tile as tile
from concourse import mybir
from concourse.tile_rust import add_dep_helper

T = int(os.environ.get("RNN_T", "512"))  # env override only for debugging
B = 2048
NCORES = 8
BC = B // NCORES          # batch per core = 256
HB = BC // 2              # half-batch = 128
VOCAB, EMB, H1, H2, H3, OUT = 27, 20, 32, 64, 64, 26
S = T + 3                 # macro steps incl. pipeline flush (FC lags 3)

MM_DT = mybir.dt.bfloat16     # matmul operand dtype (states/weights)

import ml_dtypes  # noqa: E402

_NP_OF = {mybir.dt.bfloat16: ml_dtypes.bfloat16, mybir.dt.float32: np.float32}

P1 = 64   # partition base of the pre1/h1 block


def _build_nc():
    nc = bacc.Bacc()
    f32 = mybir.dt.float32
    mdt = MM_DT

    oh_d = nc.dram_tensor("oh", [VOCAB, T * BC], mdt, kind="ExternalInput")
    la_d = nc.dram_tensor("la", [128, 128], mdt, kind="ExternalInput")
    lb_d = nc.dram_tensor("lb", [128, 128], mdt, kind="ExternalInput")
    lc_d = nc.dram_tensor("lc", [128, 128], mdt, kind="ExternalInput")
    le_d = nc.dram_tensor("le", [128, 128], mdt, kind="ExternalInput")
    lf_d = nc.dram_tensor("lf", [128, 128], mdt, kind="ExternalInput")
    b23_d = nc.dram_tensor("b23", [128, 1], f32, kind="ExternalInput")
    bfc_d = nc.dram_tensor("bfc", [OUT, 1], f32, kind="ExternalInput")
    o_d = nc.dram_tensor("o", [OUT, T * BC], f32, kind="ExternalOutput")

    with tile.TileContext(nc) as tc:
        with (
            tc.tile_pool(name="wpool", bufs=1) as wpool,
            tc.tile_pool(name="hpool", bufs=6) as hpool,
            tc.tile_pool(name="ohpool", bufs=3) as ohpool,
            tc.tile_pool(name="opool", bufs=3) as opool,
            tc.tile_pool(name="ppool", bufs=4, space="PSUM") as ppool,
            tc.tile_pool(name="fcpool", bufs=2, space="PSUM") as fcpool,
            tc.tile_pool(name="warmp", bufs=1, space="PSUM") as warmp,
        ):
            # All five stationary operands are zero-padded to the full
            # [128, 128] array: full-array matmuls without tile_position get
            # their LDWEIGHTS pipelined into the previous matmul's stream
            # (background weight buffer), so each LDW+MM pair costs only the
            # ~N/f stream time.  Padded rows multiply garbage rhs partitions
            # by zero; all garbage regions hold finite tanh outputs or zeros.
            la = wpool.tile([128, 128], mdt)   # [Whh2^T Wih3^T; 0 Whh3^T]
            lb = wpool.tile([128, 128], mdt)   # rows 64:96 = W_ih2^T | 0
            lc = wpool.tile([128, 128], mdt)   # rows 64:96 = [0 W_hh1^T 0]
            le = wpool.tile([128, 128], mdt)   # rows 0:27 = [0 EW'' 0]
            lf = wpool.tile([128, 128], mdt)   # rows 64:128 = W_fc^T | 0
            b23 = wpool.tile([128, 1], f32)
            bfc = wpool.tile([OUT, 1], f32)

            zst = wpool.tile([128, 2 * HB], mdt)   # zero initial state
            nc.vector.memset(zst[:], 0.0)
            zoh = wpool.tile([128, HB], mdt)       # zero one-hot, flush steps
            nc.vector.memset(zoh[:], 0.0)

            # One-hot rhs tiles are [128, 8*HB] (4 macro-steps per DMA) with
            # only rows 0:27 DMA'd; rows 27:128 are zeroed once per slot.
            # bufs=1 per tag: each tag is ONE buffer (a tag without bufs=1
            # would rotate `pool.bufs` buffers and the prime-once memset
            # would only ever touch the first -> uninitialized rows 27:128,
            # NaN poison on cores whose SBUF garbage contains NaNs).
            ohslots = []
            for i in range(3):
                t_ = ohpool.tile([128, 8 * HB], mdt, tag=f"oh{i}", bufs=1)
                nc.vector.memset(t_[:], 0.0)
                ohslots.append(t_)

            # DMA order: le + first one-hot chunk first (they gate the loop's
            # opener matmul), then the rest of the weights.
            nc.sync.dma_start(le[:], le_d[:])
            # Step 0 needs only the first 2*HB one-hot cols; land them as a
            # small fast DMA so the cold Sync queue's 55KB transfer doesn't
            # gate the first matmul group.
            nc.sync.dma_start(ohslots[0][0:VOCAB, 0:2 * HB], oh_d[:, 0:2 * HB])
            nc.sync.dma_start(ohslots[0][0:VOCAB, 2 * HB:8 * HB],
                              oh_d[:, 2 * HB:8 * HB])
            nc.sync.dma_start(la[:], la_d[:])
            nc.sync.dma_start(lb[:], lb_d[:])
            nc.sync.dma_start(lc[:], lc_d[:])
            nc.sync.dma_start(lf[:], lf_d[:])
            nc.sync.dma_start(b23[:], b23_d[:])
            nc.sync.dma_start(bfc[:], bfc_d[:])

            # PE warmup: back-to-back matmuls trip the HAM clock gate to 8/8
            # (2.4 GHz) and keep it there until the loop's first matmuls are
            # ready (~14us in: weight + first one-hot DMAs).  The profile of
            # the 12-MM version showed HAM re-throttling at 17.5us and the
            # whole loop running at K=4/8 (1.2 GHz).
            warm = wpool.tile([128, 512], mdt)
            nc.vector.memset(warm[:], 0.0)
            wp = warmp.tile([128, 512], mybir.dt.float32)
            for _ in range(18):
                nc.tensor.matmul(wp[:], warm[:, 0:128], warm[:], start=True, stop=True)

            # Prime the ACT tanh table set during the warmup/DMA window;
            # otherwise the first loop ACTIVATE pays a 1.3us ACT_TABLE_LOAD
            # mid-pipeline-fill (and the resulting PE idle re-throttles HAM).
            actprime = wpool.tile([1, 2], f32)
            nc.vector.memset(actprime[:], 0.0)
            nc.scalar.activation(actprime[0:1, 1:2], actprime[0:1, 0:1],
                                 mybir.ActivationFunctionType.Tanh)

            hprev = [zst, zst]
            oht = ohslots[0]
            fco = None
            fco_prev = None
            tanh = mybir.ActivationFunctionType.Tanh

            for s in range(S):
                if s % 4 == 0 and 0 < s < T:
                    g = s // 4
                    oht = ohpool.tile([128, 8 * HB], mdt, tag=f"oh{g % 3}",
                                      bufs=1)
                    nc.sync.dma_start(oht[0:VOCAB, :],
                                      oh_d[:, 8 * HB * g:8 * HB * (g + 1)])
                for half in range(2):
                    hp = hprev[half]
                    if s < T + 2:
                        p = ppool.tile([128, 2 * HB], f32)
                        if s < T:
                            o0 = (s % 4) * 2 * HB + half * HB
                            ohs = oht[:, o0:o0 + HB]
                        else:
                            ohs = zoh[:]
                        # One accumulation group per half-step+bank.  The
                        # opener (start=True) clears has_written for all 128
                        # partitions across the full bank width.  The one-hot
                        # matmul plays opener: it has no recurrence dep, so
                        # the PE streams it during the ACT wait instead of
                        # stalling head-of-line on h(s-1).
                        mm_e = nc.tensor.matmul(p[:, HB:2 * HB], le[:], ohs,
                                                start=True, stop=False,
                                                skip_group_check=True)
                        mm_la = nc.tensor.matmul(p[:, 0:HB], la[:], hp[:, 0:HB],
                                                 start=False, stop=False,
                                                 skip_group_check=True)
                        add_dep_helper(mm_la.ins, mm_e.ins, sync=False,
                                       reason="group opener executes first")
                        nc.tensor.matmul(p[:, 0:HB], lb[:], hp[:, HB:2 * HB],
                                         start=False, stop=False,
                                         skip_group_check=True)
                        nc.tensor.matmul(p[:, HB:2 * HB], lc[:],
                                         hp[:, HB:2 * HB],
                                         start=False, stop=True,
                                         skip_group_check=True)
                    # FC for t3 = s-3 reads hp (= hn of step s-1, whose h3
                    # rows carry layer-3's output for t3).  Emitted AFTER the
                    # chain matmuls: ahead of them it head-of-line blocks the
                    # PE queue on its semaphore wait (measured 616ns stalls on
                    # alternating half-steps).
                    if s >= 3:
                        t3 = s - 3
                        j = 2 * (t3 % 2) + half
                        if j == 0:
                            fco_prev = fco
                            fco = fcpool.tile([128, 4 * HB], f32)
                        nc.tensor.matmul(fco[:, HB * j:HB * (j + 1)],
                                         lf[:], hp[:, 0:HB],
                                         start=True, stop=True,
                                         skip_group_check=True)
                        if j == 3:
                            c = t3 // 2
                            outs = opool.tile([OUT, 4 * HB], f32)
                            nc.vector.tensor_scalar_add(outs[:],
                                                        fco[0:OUT, :], bfc[:])
                            nc.sync.dma_start(o_d[:, 4 * HB * c:4 * HB * (c + 1)],
                                              outs[:])
                    if s < T + 2:
                        hn = hpool.tile([128, 2 * HB], mdt)
                        nc.scalar.activation(hn[:], p[:], tanh, bias=b23[:])
                        if s == 0:
                            nc.vector.memset(hn[:, 0:HB], 0.0)      # H2,H3 inv
                        elif s == 1:
                            nc.vector.memset(hn[H2:128, 0:HB], 0.0)  # H3 inv
                        hprev[half] = hn
    nc.compile()
    return nc


_NC_CACHE = None


def _get_nc():
    global _NC_CACHE
    if _NC_CACHE is None:
        _NC_CACHE = _build_nc()
    return _NC_CACHE


def _prep_inputs(inputs):
    npdt = _NP_OF[MM_DT]
    f32 = np.float32
    x = np.asarray(inputs["x"]).astype(np.int64)            # (T, B)
    emb = np.asarray(inputs["emb"], f32)
    W_ih1 = np.asarray(inputs["W_ih1"], f32)
    W_hh1 = np.asarray(inputs["W_hh1"], f32)
    b1 = np.asarray(inputs["b_ih1"], f32) + np.asarray(inputs["b_hh1"], f32)
    W_ih2 = np.asarray(inputs["W_ih2"], f32)
    W_hh2 = np.asarray(inputs["W_hh2"], f32)
    b2 = np.asarray(inputs["b_ih2"], f32) + np.asarray(inputs["b_hh2"], f32)
    W_ih3 = np.asarray(inputs["W_ih3"], f32)
    W_hh3 = np.asarray(inputs["W_hh3"], f32)
    b3 = np.asarray(inputs["b_ih3"], f32) + np.asarray(inputs["b_hh3"], f32)
    W_fc = np.asarray(inputs["W_fc"], f32)
    b_fc = np.asarray(inputs["b_fc"], f32)

    # lhsT blocks (stationary operands), all zero-padded to [128, 128] so
    # every matmul is a full-array LDW+MM pair with pipelined weight load.
    la = np.zeros((128, 128), f32)
    la[0:H2, 0:H2] = W_hh2.T
    la[0:H2, H2:] = W_ih3.T
    la[H2:, H2:] = W_hh3.T
    lb = np.zeros((128, 128), f32)
    lb[P1:P1 + H1, 0:H2] = W_ih2.T
    lc = np.zeros((128, 128), f32)
    lc[P1:P1 + H1, P1:P1 + H1] = W_hh1.T
    # EW'' table: emb @ W_ih1^T + b1, minus the b23[64:96] (= b3[:32]) that
    # the ACT bias vector adds on the pre1 partitions.  Rows 0:27 (one-hot
    # vocab), output cols 64:96 (pre1 partitions).
    ew = emb @ W_ih1.T + b1[None, :] - b3[None, 0:H1]        # [27, 32]
    le = np.zeros((128, 128), f32)
    le[0:VOCAB, P1:P1 + H1] = ew
    lf = np.zeros((128, 128), f32)
    lf[P1:128, 0:OUT] = W_fc.T
    b23 = np.concatenate([b2, b3]).reshape(128, 1).astype(f32)
    bfc = b_fc.reshape(OUT, 1).astype(f32)

    shared = {
        "la": la.astype(npdt), "lb": lb.astype(npdt), "lc": lc.astype(npdt),
        "le": le.astype(npdt), "lf": lf.astype(npdt), "b23": b23, "bfc": bfc,
    }
    in_maps = []
    for core in range(NCORES):
        xc = x[:, core * BC:(core + 1) * BC]                 # (T, BC)
        # one-hot [27, T*BC], free order (t, b)
        oh = (xc.reshape(T * BC)[None, :] == np.arange(VOCAB)[:, None])
        in_maps.append(dict(shared, oh=np.ascontiguousarray(oh.astype(npdt))))
    return in_maps


def _assemble(results):
    cores = []
    for core in range(NCORES):
        o = results[core]["o"]                               # [26, T*BC]
        cores.append(o.reshape(OUT, T, BC).transpose(1, 2, 0))
    return np.ascontiguousarray(np.concatenate(cores, axis=1), dtype=np.float32)


def _run(inputs, **spmd_kwargs):
    """Returns (output, BassKernelResults). spmd_kwargs e.g. trace=True."""
    from concourse.bass_utils import run_bass_kernel_spmd
    nc = _get_nc()
    in_maps = _prep_inputs(inputs)
    res = run_bass_kernel_spmd(nc, in_maps, core_ids=list(range(NCORES)),
                               **spmd_kwargs)
    return _assemble(res.results), res


def kernel(**inputs) -> np.ndarray:
    return _run(inputs)[0]


if __name__ == "__main__":
    import reference as R
    ins = {k: np.asarray(v) for k, v in R.setup_inputs().items()}
    got = kernel(**ins)
    import jax.numpy as jnp
    want = np.asarray(R.reference(**{k: jnp.asarray(v) for k, v in ins.items()}))
    err = np.abs(got - want)
    print("absmax", err.max(), "rel", err.max() / np.abs(want).max())

